# revision 17
# baseline (speedup 1.0000x reference)
"""GQA attention block (RoPE + causal attention + output proj) on 8 TRN2 NeuronCores.

Sharding: batch (B=2) x kv-head-group (KVH=4) -> 8 cores.
Core c handles batch b=c//4, kv group g=c%4 (q heads 4g..4g+3, kv head g).
Per-core tensor-parallel attention; AllGather of per-head outputs within each
batch's 4-core group; column-split wo after the gather.

v2: fully software-pipelined spans. Q/KV projection + RoPE for span J+1 and
the output projection for earlier spans are interleaved into span J's
attention stream as PE filler, so the TensorEngine never idles (keeps the
p-state ramp at max clock). The softmax normalization chain runs entirely off
the PE (DVE reciprocal -> GpSimd partition broadcast -> DVE scale). A tiny
warmup AllGather absorbs inter-core startup skew before the real collectives.
Span 3's gather is split per head-pair so the final collective overlaps the
remaining output projections. Output is stored bf16 and upcast on host.
"""

import sys

if "/opt/trn_rl_repo" not in sys.path:
    sys.path.insert(0, "/opt/trn_rl_repo")

import numpy as np
import ml_dtypes

import concourse.bass as bass
import concourse.mybir as mybir
import concourse.tile as tile
from concourse import bacc
from concourse.bass_utils import run_bass_kernel_spmd

BF16 = ml_dtypes.bfloat16

B, S, HID = 2, 2048, 1024
H, KVH, D = 16, 4, 64
G = H // KVH
N_CORES = 8
SPAN = 512
NSPAN = S // SPAN  # 4
NCH = HID // 128  # 8 contraction chunks
F32 = mybir.dt.float32
BF = mybir.dt.bfloat16

TRACE = False
_CACHED = {}


def _build_nc():
    nc = bacc.Bacc("TRN2", target_bir_lowering=False, debug=False, num_devices=N_CORES)

    xT = nc.dram_tensor("xT", [HID, S], BF, kind="ExternalInput")
    wq = nc.dram_tensor("wq", [HID, 256], BF, kind="ExternalInput")
    wkv = nc.dram_tensor("wkv", [HID, 128], BF, kind="ExternalInput")
    wo = nc.dram_tensor("wo", [HID, 256], BF, kind="ExternalInput")
    c2 = nc.dram_tensor("c2", [128, S], BF, kind="ExternalInput")
    s2 = nc.dram_tensor("s2", [128, S], BF, kind="ExternalInput")
    c1 = nc.dram_tensor("c1", [64, S], BF, kind="ExternalInput")
    s1 = nc.dram_tensor("s1", [64, S], BF, kind="ExternalInput")
    ident = nc.dram_tensor("ident", [128, 128], BF, kind="ExternalInput")
    dmask = nc.dram_tensor("dmask", [128, 128], BF, kind="ExternalInput")
    out = nc.dram_tensor("out", [256, S], BF, kind="ExternalOutput")

    EXP = mybir.ActivationFunctionType.Exp
    rg = [[0, 1, 2, 3], [4, 5, 6, 7]]

    with tile.TileContext(nc) as tc:
        with (
            tc.tile_pool(name="main", bufs=1) as main,
            tc.tile_pool(name="dramp", bufs=1, space="DRAM") as dramp,
            tc.tile_pool(name="flex", bufs=2, space="PSUM") as flexp,
            tc.tile_pool(name="psO", bufs=1, space="PSUM") as psO,
            tc.tile_pool(name="ropep", bufs=2) as ropep,
            tc.tile_pool(name="pp", bufs=9) as pp,
            tc.tile_pool(name="work", bufs=2) as work,
        ):
            # ---- persistent SBUF ----
            xT_sb = main.tile([128, NCH, S], BF, name="xT_sb")
            wq_sb = main.tile([128, NCH, 256], BF, name="wq_sb")
            wkv_sb = main.tile([128, NCH, 128], BF, name="wkv_sb")
            wo_sb = main.tile([128, NCH, 256], BF, name="wo_sb")
            c2_sb = main.tile([128, S], BF, name="c2_sb")
            s2_sb = main.tile([128, S], BF, name="s2_sb")
            c1_sb = main.tile([64, S], BF, name="c1_sb")
            s1_sb = main.tile([64, S], BF, name="s1_sb")
            ident_sb = main.tile([128, 128], BF, name="ident_sb")
            dmask_sb = main.tile([128, 128], BF, name="dmask_sb")
            qT0_sb = main.tile([128, S], BF, name="qT0_sb")
            qT1_sb = main.tile([128, S], BF, name="qT1_sb")
            kT2_sb = main.tile([128, S], BF, name="kT2_sb")
            vT_sb = main.tile([64, S], BF, name="vT_sb")
            vaug_sb = main.tile([128, S // 128, 65], BF, name="vaug_sb")
            wsrc_sb = main.tile([128, 16], BF, name="wsrc_sb")
            wscr_sb = main.tile([1, 16], BF, name="wscr_sb")
            ones_sb = main.tile([1, 64], BF, name="ones_sb")
            qT_sb = [qT0_sb, qT1_sb]

            # DRAM staging for collectives
            agin = [dramp.tile([256, SPAN], BF, name=f"agin{J}") for J in range(4)]
            agout = [
                dramp.tile([4 * 256, SPAN], BF, name=f"agout{J}") for J in range(4)
            ]

            # ---- prologue: input DMAs (priority order) ----
            nc.vector.memset(wsrc_sb[:], 1.0)
            nc.vector.memset(vaug_sb[:, :, 64:65], 1.0)
            nc.vector.memset(ones_sb[:], 1.0)
            # SP queue: wkv, x spans (in need order), wo
            nc.sync.dma_start(
                wkv_sb[:], wkv[:].rearrange("(k p) c -> p k c", p=128)
            )
            for sp_i in range(NSPAN):
                s0 = SPAN * sp_i
                nc.sync.dma_start(
                    xT_sb[:, :, s0 : s0 + SPAN],
                    xT[:, s0 : s0 + SPAN].rearrange("(k p) c -> p k c", p=128),
                )
            # ACT queue: wq, rope tables, ident, dmask (all needed early-ish)
            nc.sync.dma_start(
                wq_sb[:], wq[:].rearrange("(k p) c -> p k c", p=128)
            )
            nc.sync.dma_start(c1_sb[:], c1[:])
            nc.sync.dma_start(s1_sb[:], s1[:])
            nc.sync.dma_start(ident_sb[:], ident[:])
            nc.sync.dma_start(c2_sb[:], c2[:])
            nc.sync.dma_start(s2_sb[:], s2[:])
            nc.sync.dma_start(dmask_sb[:], dmask[:])
            # preload the Exp activation table while DMAs stream
            nc.scalar.activation(wscr_sb[:], wsrc_sb[0:1, :], EXP)
            # SP: wo last of the inputs
            nc.sync.dma_start(
                wo_sb[:], wo[:].rearrange("(k p) c -> p k c", p=128)
            )

            # ---- per-span projection / rope / transpose emitters ----
            def emit_kv_proj(I):
                s0 = SPAN * I
                kvp = flexp.tile([128, SPAN], F32, tag="s", name=f"kvp{I}")
                for k in range(NCH):
                    nc.tensor.matmul(
                        kvp[:],
                        wkv_sb[:, k, :],
                        xT_sb[:, k, s0 : s0 + SPAN],
                        start=(k == 0),
                        stop=(k == NCH - 1),
                    )
                kb = ropep.tile([64, SPAN], BF, tag="kb", name=f"kb{I}")
                nc.vector.tensor_copy(kb[:], kvp[0:64, :])
                nc.vector.tensor_copy(vT_sb[:, s0 : s0 + SPAN], kvp[64:128, :])
                tcosk = ropep.tile([64, SPAN], BF, tag="tcos", name=f"tcosk{I}")
                tsink = ropep.tile([64, SPAN], BF, tag="tsin", name=f"tsink{I}")
                nc.vector.tensor_mul(tcosk[:], kb[:], c1_sb[:, s0 : s0 + SPAN])
                for dst, src in ((0, 32), (32, 0)):
                    nc.vector.tensor_mul(
                        tsink[dst : dst + 32, :],
                        kb[src : src + 32, :],
                        s1_sb[src : src + 32, s0 : s0 + SPAN],
                    )
                nc.vector.tensor_add(
                    kT2_sb[0:64, s0 : s0 + SPAN], tcosk[:], tsink[:]
                )
                nc.vector.tensor_copy(
                    kT2_sb[64:128, s0 : s0 + SPAN], kT2_sb[0:64, s0 : s0 + SPAN]
                )

            def emit_vtrans(I):
                trp = flexp.tile([128, 4, 64], BF, tag="s", name=f"tr{I}")
                for tt in range(4):
                    t = 4 * I + tt
                    nc.tensor.transpose(
                        trp[:, tt, :],
                        vT_sb[:, 128 * t : 128 * (t + 1)],
                        ident_sb[0:64, 0:64],
                    )
                nc.vector.tensor_copy(vaug_sb[:, 4 * I : 4 * I + 4, 0:64], trp[:])

            qp_state = {}

            def emit_q_proj_half(I, p):
                s0 = SPAN * I
                if p == 0:
                    qp_state[I] = flexp.tile(
                        [128, 2, SPAN], F32, tag="s", name=f"qp{I}"
                    )
                qp = qp_state[I]
                for k in range(NCH):
                    nc.tensor.matmul(
                        qp[:, p, :],
                        wq_sb[:, k, 128 * p : 128 * (p + 1)],
                        xT_sb[:, k, s0 : s0 + SPAN],
                        start=(k == 0),
                        stop=(k == NCH - 1),
                    )
                qb = ropep.tile([128, SPAN], BF, tag="qb", name=f"qb{I}_{p}")
                nc.scalar.copy(qb[:], qp[:, p, :])
                tcos = ropep.tile([128, SPAN], BF, tag="tcos", name=f"tc{I}_{p}")
                tsin = ropep.tile([128, SPAN], BF, tag="tsin", name=f"ts{I}_{p}")
                nc.vector.tensor_mul(tcos[:], qb[:], c2_sb[:, s0 : s0 + SPAN])
                for dst, src in ((0, 32), (32, 0), (64, 96), (96, 64)):
                    nc.vector.tensor_mul(
                        tsin[dst : dst + 32, :],
                        qb[src : src + 32, :],
                        s2_sb[src : src + 32, s0 : s0 + SPAN],
                    )
                nc.vector.tensor_add(
                    qT_sb[p][:, s0 : s0 + SPAN], tcos[:], tsin[:]
                )

            # ---- normalization + gather + output projection emitters ----
            def emit_norm(J, pr, opsum, agin_t, row0):
                dsb = work.tile([1, 2 * SPAN], BF, tag="dsb", name=f"dsb{J}_{pr}")
                nc.vector.tensor_copy(dsb[:], opsum[64:65, :, :])
                bc = flexp.tile([64, 2, SPAN], F32, tag="s", name=f"bc{J}_{pr}")
                for hh in range(2):
                    nc.tensor.matmul(
                        bc[:, hh, :],
                        ones_sb[:],
                        dsb[0:1, SPAN * hh : SPAN * (hh + 1)],
                        start=True,
                        stop=True,
                    )
                rec = work.tile([64, 2, SPAN], F32, tag="rec", name=f"rec{J}_{pr}")
                nc.vector.reciprocal_approx_fast(rec[:], bc[:])
                onrm = work.tile([64, 2, SPAN], BF, tag="onrm", name=f"on{J}_{pr}")
                nc.vector.tensor_mul(onrm[:], opsum[0:64, :, :], rec[:])
                nc.sync.dma_start(
                    agin_t[row0 : row0 + 128, :].rearrange(
                        "(hh p) c -> p hh c", hh=2
                    ),
                    onrm[:],
                )

            ofull_t = {}

            def emit_gather(J):
                nc.gpsimd.collective_compute(
                    "AllGather",
                    mybir.AluOpType.bypass,
                    replica_groups=rg,
                    ins=[agin[J][:].opt()],
                    outs=[agout[J][:].opt()],
                )
                of = work.tile([128, NCH, SPAN], BF, tag="ofull", name=f"of{J}")
                nc.sync.dma_start(
                    of[:], agout[J][:].rearrange("(k p) c -> p k c", p=128)
                )
                ofull_t[J] = of

            po_state = {}

            def emit_oproj_half(J, half):
                q0 = SPAN * J
                if half == 0:
                    po_state[J] = flexp.tile(
                        [128, 2, SPAN], F32, tag="s", name=f"po{J}"
                    )
                po = po_state[J]
                of = ofull_t[J]
                for k in range(NCH):
                    nc.tensor.matmul(
                        po[:, half, :],
                        wo_sb[:, k, 128 * half : 128 * (half + 1)],
                        of[:, k, :],
                        start=(k == 0),
                        stop=(k == NCH - 1),
                    )
                if half == 1:
                    outT = work.tile(
                        [128, 2, SPAN], BF, tag="outT", name=f"ot{J}"
                    )
                    nc.vector.tensor_copy(outT[:], po[:])
                    nc.sync.dma_start(
                        out[:, q0 : q0 + SPAN].rearrange(
                            "(hh p) c -> p hh c", hh=2
                        ),
                        outT[:],
                    )

            # ---- filler schedule: (J, pr, batch_idx) -> [closures] ----
            fillers = {
                (0, 0, 1): [lambda: emit_kv_proj(1)],
                (0, 1, 0): [lambda: emit_q_proj_half(1, 0)],
                (0, 1, 1): [lambda: emit_q_proj_half(1, 1)],
                (1, 0, 0): [lambda: emit_vtrans(1)],
                (1, 0, 1): [lambda: emit_kv_proj(2)],
                (1, 1, 0): [lambda: emit_q_proj_half(2, 0)],
                (1, 1, 1): [lambda: emit_q_proj_half(2, 1)],
                (2, 0, 0): [lambda: emit_vtrans(2)],
                (2, 0, 1): [lambda: emit_oproj_half(0, 0)],
                (2, 0, 2): [lambda: emit_oproj_half(0, 1)],
                (2, 0, 3): [lambda: emit_kv_proj(3)],
                (2, 1, 0): [lambda: emit_q_proj_half(3, 0)],
                (2, 1, 1): [lambda: emit_q_proj_half(3, 1)],
                (3, 0, 0): [lambda: emit_vtrans(3)],
                (3, 0, 2): [lambda: emit_oproj_half(1, 0)],
                (3, 0, 4): [lambda: emit_oproj_half(1, 1)],
                (3, 1, 2): [lambda: emit_oproj_half(2, 0)],
                (3, 1, 5): [lambda: emit_oproj_half(2, 1)],
            }

            # ---- prologue projections for span 0 ----
            emit_kv_proj(0)
            emit_vtrans(0)
            emit_q_proj_half(0, 0)
            emit_q_proj_half(0, 1)

            # ---- attention spans ----
            for J in range(NSPAN):
                q0 = SPAN * J
                nk = 4 * (J + 1)
                for pr in range(2):
                    opsum = psO.tile(
                        [128, 2, SPAN], F32, tag=f"o{pr}", name=f"opsum{J}_{pr}"
                    )
                    src = qT_sb[pr]
                    pv_queue = []

                    def emit_pv(j, pt, off, opsum=opsum, nk=nk):
                        for hh in range(2):
                            nc.tensor.matmul(
                                opsum[0:65, hh, off:SPAN],
                                vaug_sb[:, j, :],
                                pt[:, hh, off:SPAN],
                                start=(j == 0),
                                stop=(j == nk - 1),
                            )

                    for bi in range(nk // 2):
                        batch = []
                        for j in range(2 * bi, 2 * bi + 2):
                            jj = j - 4 * J
                            off = 128 * jj if jj > 0 else 0
                            sps = flexp.tile(
                                [128, 2, SPAN], F32, tag="s", name=f"s{J}_{j}_{pr}"
                            )
                            for hh in range(2):
                                nc.tensor.matmul(
                                    sps[:, hh, off:SPAN],
                                    kT2_sb[
                                        64 * hh : 64 * (hh + 1),
                                        128 * j : 128 * (j + 1),
                                    ],
                                    src[
                                        64 * hh : 64 * (hh + 1),
                                        q0 + off : q0 + SPAN,
                                    ],
                                    start=True,
                                    stop=True,
                                )
                            batch.append((j, sps, off))
                        for j, sps, off in batch:
                            pt = pp.tile(
                                [128, 2, SPAN], BF, tag="p", name=f"p{J}_{j}_{pr}"
                            )
                            nc.scalar.activation(
                                pt[:, :, off:SPAN], sps[:, :, off:SPAN], EXP
                            )
                            jj = j - 4 * J
                            if jj >= 0:
                                for hh in range(2):
                                    nc.vector.tensor_mul(
                                        pt[:, hh, off : off + 128],
                                        pt[:, hh, off : off + 128],
                                        dmask_sb[:],
                                    )
                            pv_queue.append((j, pt, off))
                        while len(pv_queue) > 4:
                            emit_pv(*pv_queue.pop(0))
                            emit_pv(*pv_queue.pop(0))
                        for fn in fillers.get((J, pr, bi), ()):
                            fn()
                    for args in pv_queue:
                        emit_pv(*args)

                    # normalization (off-PE) + gather
                    emit_norm(J, pr, opsum, agin[J], 128 * pr)
                    if pr == 1:
                        emit_gather(J)

                # end of span J
            # ---- tail: remaining output projections ----
            emit_oproj_half(3, 0)
            emit_oproj_half(3, 1)

    nc.finalize()
    return nc


def _host_inputs(x, cos, sin, wq, wk, wv, wo):
    cosT = np.ascontiguousarray(cos.T).astype(np.float32)  # [64, S]
    sinT = np.ascontiguousarray(sin.T).astype(np.float32)
    s1n = np.concatenate([-sinT[0:32], sinT[32:64]], axis=0)  # [64, S]
    c2n = np.concatenate([cosT, cosT], axis=0).astype(BF16)  # [128, S]
    # partition-swapped: row p holds the sin factor for the partner row p^32,
    # so both DVE operands read from the same base partition
    s1w = np.concatenate([s1n[32:64], s1n[0:32]], axis=0)
    s2w = np.concatenate([s1w, s1w], axis=0).astype(BF16)
    cosT = cosT.astype(BF16)
    s1w = s1w.astype(BF16)
    ident = np.eye(128, dtype=BF16)
    # upper-triangular (incl diagonal) keep-mask for the causal boundary block
    dmaskh = (np.arange(128)[None, :] >= np.arange(128)[:, None]).astype(BF16)

    in_maps = []
    for c in range(N_CORES):
        b, g = c // 4, c % 4
        xT = np.ascontiguousarray(x[b].T).astype(BF16)
        wq_c = np.ascontiguousarray(wq[:, 256 * g : 256 * (g + 1)] / 8.0).astype(BF16)
        wkv_c = np.ascontiguousarray(
            np.concatenate(
                [wk[:, 64 * g : 64 * (g + 1)], wv[:, 64 * g : 64 * (g + 1)]], axis=1
            )
        ).astype(BF16)
        wo_c = np.ascontiguousarray(wo[:, 256 * g : 256 * (g + 1)]).astype(BF16)
        in_maps.append(
            {
                "xT": xT,
                "wq": wq_c,
                "wkv": wkv_c,
                "wo": wo_c,
                "c2": c2n,
                "s2": s2w,
                "c1": cosT,
                "s1": s1w,
                "ident": ident,
                "dmask": dmaskh,
            }
        )
    return in_maps


def kernel(x, cos, sin, wq, wk, wv, wo):
    if "nc" not in _CACHED:
        _CACHED["nc"] = _build_nc()
    nc = _CACHED["nc"]
    in_maps = _host_inputs(
        np.asarray(x, np.float32),
        np.asarray(cos, np.float32),
        np.asarray(sin, np.float32),
        np.asarray(wq, np.float32),
        np.asarray(wk, np.float32),
        np.asarray(wv, np.float32),
        np.asarray(wo, np.float32),
    )
    res = run_bass_kernel_spmd(
        nc, in_maps, core_ids=list(range(N_CORES)), trace=TRACE
    )
    _CACHED["last_result"] = res
    out = np.empty((B, S, HID), dtype=np.float32)
    for c in range(N_CORES):
        b, g = c // 4, c % 4
        out[b, :, 256 * g : 256 * (g + 1)] = res.results[c]["out"].astype(np.float32).T
    return out
